# revision 22
# baseline (speedup 1.0000x reference)
"""Multi-head attention (B=4, N=2048, C=1024, H=16, hd=64) on 8 TRN2 cores.

Sharding: tensor-parallel over heads — core c owns heads (2c, 2c+1), i.e.
channel slice [128c, 128c+128) of the concat-head space. Each core computes
q/k/v for its 2 heads over the full batch, attention, and a partial output
projection (contraction over its 128 channels). Host sums the 8 partials.

Per-core device pipeline (per batch b):
  1. qT/kT/vT [128, N] = W_loc.T @ x.T   (x.T supplied pre-transposed by host)
  2. v_ones [N(j), 65] per head via PE transpose of vT; col 64 = ones
  3. per 512-query chunk: scoresT [j, i] = kT_tile.T @ qT (heads row-packed),
     exp on ACT (scale folded), AV: [v | 1].T @ exp(scoresT) accumulated over
     j — row 64 of the PSUM result is the softmax denominator; DVE normalizes
     into outT [128, N]
  4. y_partial = outT.T @ proj_w[slice]  → DRAM

Softmax skips the max-subtraction: scores/8 ~ N(0,1), max over 2048 ≈ 4, so
exp is comfortably in range and the result is mathematically identical.
"""

import numpy as np

import concourse.bass as bass
import concourse.mybir as mybir
import concourse.tile as tile
from concourse import bass_utils

P = 128
F32 = mybir.dt.float32
F32R = mybir.dt.float32r
BF16 = mybir.dt.bfloat16


_SPLIT_WAIT_TYPES = {
    "InstMatmult", "InstLdweights", "InstTensorScalarPtr", "InstActivation",
    "InstTensorTensor", "InstTensorCopy", "InstReciprocal", "InstMemSet",
    "InstStreamTranspose", "InstCopy", "InstTensorScalar", "InstDMACopy",
    "InstDmaTransposeAnt", "InstDrain", "InstNoOp", "InstIota", "InstEventSemaphore",
}


def _split_matmul_waits(nc):
    """Several walrus engine-instruction structs accept only one sync wait.
    Move excess waits onto injected same-engine NoOps just before the
    instruction — semantics are identical (engine executes in program order)."""
    import bass_rust
    for f in nc.m.functions:
        for blk in f.blocks:
            out = []
            for ins in blk.instructions:
                si = ins.sync_info
                if (si is not None and len(si.on_wait) >= 2
                        and type(ins).__name__ in _SPLIT_WAIT_TYPES):
                    waits = list(si.on_wait)
                    for i, w in enumerate(waits[:-1]):
                        nop = mybir.InstNoOp(name=f"{ins.name}_w{i}", ins=[], outs=[])
                        nop.engine = ins.engine
                        nop.sync_info = bass_rust.SyncInfo(on_wait=[w], on_update=[])
                        out.append(nop)
                    ins.sync_info = bass_rust.SyncInfo(
                        on_wait=[waits[-1]], on_update=list(si.on_update))
                out.append(ins)
            blk.instructions = out


def build_program(nc, B, N, C, n_cores=8, attn_dt=BF16):
    """Emit the per-core SPMD program. Head-local dims: 2 heads x 64 = 128."""
    CS = C // P          # contraction subtiles for qkv projection
    NT = N // P          # 128-row tiles along sequence
    IC = 512             # query-chunk (f32 PSUM bank limit)
    NIC = N // IC
    JT = NT              # key tiles
    HL, HD = 2, 64       # heads per core, head dim
    CO = C               # proj output channels

    xt = nc.dram_tensor("xt", [B, CS, P, N], F32R, kind="ExternalInput").ap()
    wq = nc.dram_tensor("wq", [P, CS, P], F32R, kind="ExternalInput").ap()
    wk = nc.dram_tensor("wk", [P, CS, P], F32R, kind="ExternalInput").ap()
    wv = nc.dram_tensor("wv", [P, CS, P], F32R, kind="ExternalInput").ap()
    bqkv = nc.dram_tensor("bqkv", [P, 3], F32, kind="ExternalInput").ap()
    idd = nc.dram_tensor("idd", [P, HD], F32, kind="ExternalInput").ap()
    wp = nc.dram_tensor("wp", [P, CO], F32R, kind="ExternalInput").ap()
    y = nc.dram_tensor("y", [B, N, CO], F32, kind="ExternalOutput").ap()

    with tile.TileContext(nc) as tc:
        with (
            tc.tile_pool(name="consts", bufs=1) as consts,
            tc.tile_pool(name="xtp", bufs=CS) as xtp,
            tc.tile_pool(name="qk", bufs=1) as qkp,
            tc.tile_pool(name="vo", bufs=3) as vop,
            tc.tile_pool(name="at", bufs=2 * JT + 6) as atp,
            tc.tile_pool(name="outs", bufs=2) as outp,
            tc.tile_pool(name="small", bufs=4) as smallp,
            tc.tile_pool(name="yst", bufs=4) as ystp,
            tc.tile_pool(name="psum", bufs=1, space="PSUM") as psum,
        ):
            wq_s = consts.tile([P, CS, P], F32R, tag="wq")
            wk_s = consts.tile([P, CS, P], F32R, tag="wk")
            wv_s = consts.tile([P, CS, P], F32R, tag="wv")
            wp_s = consts.tile([P, CO], F32R, tag="wp")
            bias = consts.tile([P, 3], F32, tag="bias")
            ident = consts.tile([P, HD], F32, tag="ident")
            nc.sync.dma_start(wq_s, wq)
            nc.sync.dma_start(wk_s, wk)
            nc.sync.dma_start(wv_s, wv)
            nc.sync.dma_start(wp_s, wp)
            nc.sync.dma_start(bias, bqkv)
            nc.sync.dma_start(ident, idd)

            for b in range(B):
                # ---- qT / kT / vT : [128 dims, N] ----
                xts = []
                for cs in range(CS):
                    xt_t = xtp.tile([P, N], F32R, tag="xt")
                    nc.sync.dma_start(xt_t, xt[b, cs])
                    xts.append(xt_t)

                qT = qkp.tile([P, N], F32R, tag="q")
                kT = qkp.tile([P, N], F32R, tag="k")
                vT = qkp.tile([P, N], F32, tag="v")
                for w_s, dst, bc in ((wq_s, qT, 0), (wk_s, kT, 1), (wv_s, vT, 2)):
                    for nck in range(NIC):
                        ps = psum.tile([P, IC], F32, tag="ps_qkv", bufs=2)
                        for cs in range(CS):
                            nc.tensor.matmul(
                                ps,
                                lhsT=w_s[:, cs, :],
                                rhs=xts[cs][:, nck * IC:(nck + 1) * IC],
                                start=(cs == 0),
                                stop=(cs == CS - 1),
                            )
                        nc.vector.tensor_scalar_add(
                            out=dst[:, nck * IC:(nck + 1) * IC],
                            in0=ps,
                            scalar1=bias[:, bc:bc + 1],
                        )

                # ---- v_ones per head: [j(128) x JT, HD+1] ----
                vos = []
                for h in range(HL):
                    vo = vop.tile([P, JT, 2 * HD], attn_dt, tag="vo")
                    nc.vector.memset(vo[:, :, HD:2 * HD], 1.0)
                    for jt in range(JT):
                        pst = psum.tile([P, HD], F32, tag="ps_tr", bufs=1)
                        nc.tensor.transpose(
                            pst,
                            vT[h * HD:(h + 1) * HD, jt * P:(jt + 1) * P],
                            ident[h * HD:(h + 1) * HD, :],
                        )
                        nc.vector.tensor_copy(out=vo[:, jt, 0:HD], in_=pst)
                    vos.append(vo)

                outT = outp.tile([P, N], F32R, tag="outT")

                # ---- attention over query chunks ----
                for icx in range(NIC):
                    isl = slice(icx * IC, (icx + 1) * IC)
                    ats = [[None] * JT for _ in range(HL)]
                    for jt in range(JT):
                        for h in range(HL):
                            hs = slice(h * HD, (h + 1) * HD)
                            ps_s = psum.tile([P, IC], F32, tag="ps_sc", bufs=2)
                            nc.tensor.matmul(
                                ps_s,
                                lhsT=kT[hs, jt * P:(jt + 1) * P],
                                rhs=qT[hs, isl],
                                start=True,
                                stop=True,
                            )
                            at = atp.tile([P, IC], attn_dt, tag="at")
                            nc.scalar.activation(
                                out=at, in_=ps_s,
                                func=mybir.ActivationFunctionType.Exp,
                                scale=float(HD) ** -0.5,
                            )
                            ats[h][jt] = at
                    for h in range(HL):
                        ps_a = psum.tile([P, IC], F32, tag="ps_av", bufs=2)
                        for jt in range(JT):
                            nc.tensor.matmul(
                                ps_a,
                                lhsT=vos[h][:, jt, :],
                                rhs=ats[h][jt],
                                start=(jt == 0),
                                stop=(jt == JT - 1),
                            )
                        rcb = smallp.tile([HD, IC], F32, tag="rcb")
                        nc.vector.reciprocal(rcb, ps_a[HD:2 * HD, :])
                        nc.vector.tensor_tensor(
                            out=outT[h * HD:(h + 1) * HD, isl],
                            in0=ps_a[0:HD, :],
                            in1=rcb,
                            op=mybir.AluOpType.mult,
                        )

                # ---- partial projection: y[b] += outT.T @ wp ----
                for nt in range(NT):
                    for cc in range(CO // IC):
                        ps_p = psum.tile([P, IC], F32, tag="ps_proj", bufs=1)
                        nc.tensor.matmul(
                            ps_p,
                            lhsT=outT[:, nt * P:(nt + 1) * P],
                            rhs=wp_s[:, cc * IC:(cc + 1) * IC],
                            start=True,
                            stop=True,
                        )
                        ysb = ystp.tile([P, IC], F32, tag="ysb")
                        nc.vector.tensor_copy(ysb, ps_p)
                        nc.sync.dma_start(
                            y[b, nt * P:(nt + 1) * P, cc * IC:(cc + 1) * IC], ysb
                        )
    _split_matmul_waits(nc)
    return nc


def build_null_program(nc, B, N, C):
    """Same I/O signature as build_program but near-zero device work — used
    by test.py to subtract host/transfer overhead from wall-clock timing."""
    CS = C // P
    HD = 64
    nc.dram_tensor("xt", [B, CS, P, N], F32R, kind="ExternalInput").ap()
    nc.dram_tensor("wq", [P, CS, P], F32R, kind="ExternalInput").ap()
    nc.dram_tensor("wk", [P, CS, P], F32R, kind="ExternalInput").ap()
    nc.dram_tensor("wv", [P, CS, P], F32R, kind="ExternalInput").ap()
    bqkv = nc.dram_tensor("bqkv", [P, 3], F32, kind="ExternalInput").ap()
    nc.dram_tensor("idd", [P, HD], F32, kind="ExternalInput").ap()
    nc.dram_tensor("wp", [P, CO := C], F32R, kind="ExternalInput").ap()
    y = nc.dram_tensor("y", [B, N, CO], F32, kind="ExternalOutput").ap()
    with tile.TileContext(nc) as tc:
        with tc.tile_pool(name="t", bufs=1) as pool:
            t = pool.tile([P, 3], F32)
            nc.sync.dma_start(t, bqkv)
            nc.sync.dma_start(y[0, 0:P, 0:3], t)
    _split_matmul_waits(nc)
    return nc


def make_in_maps(x, qkv_w, qkv_b, proj_w, n_cores=8):
    B, N, C = x.shape
    CS = C // P
    xt = np.ascontiguousarray(x.transpose(0, 2, 1)).reshape(B, CS, P, N)
    in_maps = []
    for c in range(n_cores):
        c0 = c * P
        def wslice(off):
            w = qkv_w[:, off + c0:off + c0 + P]
            return np.ascontiguousarray(w.reshape(CS, P, P).transpose(1, 0, 2))
        in_maps.append({
            "xt": xt,
            "wq": wslice(0),
            "wk": wslice(C),
            "wv": wslice(2 * C),
            "bqkv": np.ascontiguousarray(np.stack(
                [qkv_b[c0:c0 + P], qkv_b[C + c0:C + c0 + P],
                 qkv_b[2 * C + c0:2 * C + c0 + P]], axis=1)),
            "idd": np.tile(np.eye(64, dtype=np.float32), (2, 1)),
            "wp": np.ascontiguousarray(proj_w[c0:c0 + P, :]),
        })
    return in_maps


_CACHE = {}


def _get_program(B, N, C):
    key = (B, N, C)
    if key not in _CACHE:
        nc = bass.Bass("TRN2", debug=False)
        build_program(nc, B, N, C)
        _CACHE[key] = nc
    return _CACHE[key]


def kernel(x, qkv_w, qkv_b, proj_w, proj_b, trace=False):
    x = np.asarray(x, dtype=np.float32)
    qkv_w = np.asarray(qkv_w, dtype=np.float32)
    qkv_b = np.asarray(qkv_b, dtype=np.float32)
    proj_w = np.asarray(proj_w, dtype=np.float32)
    proj_b = np.asarray(proj_b, dtype=np.float32)
    B, N, C = x.shape
    n_cores = 8

    nc = _get_program(B, N, C)
    in_maps = make_in_maps(x, qkv_w, qkv_b, proj_w, n_cores)
    res = bass_utils.run_bass_kernel_spmd(
        nc, in_maps, core_ids=list(range(n_cores)), trace=trace
    )
    acc = res.results[0]["y"].astype(np.float32, copy=True)
    for rres in res.results[1:]:
        acc += rres["y"]
    acc += proj_b
    kernel.last_results = res
    return acc
